# revision 1
# baseline (speedup 1.0000x reference)
"""KoLeoLoss Trainium2 kernel (nn_KoLeoLoss_73538430042938).

Math: rows are L2-normalized, so for the nearest neighbor j of row i (by max
cosine sim m_i), the pairwise distance is ||xn_i - xn_j|| = sqrt(2 - 2*m_i).
The device only needs, per row, the max off-diagonal entry of the normalized
Gram matrix -- no argmax indices, no gather.

Normalization factorization: with raw transposed operand XT and
rinv_i = 1/||x_i||, the kernel forms XnT = XT * rbc (rbc = rinv broadcast
across partitions, built ON DEVICE via a tiny PE transpose + one-hot
matmuls), so G = XnT.T @ XnT is the fully normalized cosine Gram and the
row-max follows directly; the diagonal gets -30000*I accumulated by one
extra PE matmul per row tile before the reduce.

Sharding/layout: data parallel over B=32 -> 4 batches/core on 8 cores. The
host ships each shard twice in bf16: row-major (for the norm pass) and
pre-transposed (the matmul operand layout) -- layout-only preprocessing,
zero FLOPs. Each core returns its [128, 32] row-max matrix; the host
applies the tiny scalar log/mean tail in float64 (mean is permutation
invariant, so no reassembly mapping is needed).

Device pipeline per batch b (N=1024 rows, D=512 dims, P=128):
  1. DMA xb [128,8,512] (row tiles) and xt [128,4,1024] (K-chunks).
  2. ssq via ScalarE Square+accum_out; nrm = sqrt(ssq/4096) (ScalarE,
     the scale is a leftover from the fp8 experiments and is undone on
     the host); rinv = 1/nrm (VectorE reciprocal).
  3. rbc broadcast: PE-transpose rinv -> [8,128], then 8 K=8 matmuls with
     one-hot stationaries replicate row t across partitions -> [128,1024].
  4. xnT[k] = xt[k] * rbc on the DVE (bf16 2x mode).
  5. Per row-tile t: G[128,1024] = sum_k xnT[k,t-slice].T @ xnT[k,half]
     (bf16, fp32 PSUM, 3 G buffers) + the diag-mask matmul, then a DVE
     reduce_max into maxes[:, b*8+t].

Scheduling: prep for batch b+1/b+2 is emitted at fixed slots inside batch
b's matmul/reduce phase so every engine FIFO stays dense; dummy warm-up
matmuls cover the PE-idle head because the HAM clock gate holds the PE at
1.2 GHz until it sees ~3.4us of sustained activity (and re-throttles after
an idle window). A dummy Sqrt pins the one ACT table set used; mixing sets
costs 1.3us per swap plus serialization.
"""

import sys

import numpy as np

_TRN = "/opt/trn_rl_repo"
if _TRN not in sys.path:
    sys.path.insert(0, _TRN)

B, N, D = 32, 1024, 512
NCORES = 8
BLOC = B // NCORES  # batches per core
P = 128
NT = N // P  # row tiles per batch
KC = D // P  # contraction chunks
NEG = -30000.0
EPS = 1e-8

_CACHE = {}


def build_nc():
    import concourse.bacc as bacc
    import concourse.mybir as mybir
    from concourse import masks, tile
    from concourse.tile_rust import add_dep_helper

    f32 = mybir.dt.float32
    bf16 = mybir.dt.bfloat16
    fp8 = mybir.dt.float8e4
    AF = mybir.ActivationFunctionType
    ALU = mybir.AluOpType

    nc = bacc.Bacc(
        "TRN2", target_bir_lowering=False, debug=False, num_devices=NCORES
    )
    xb_dram = nc.dram_tensor("xb", [BLOC, N, D], bf16, kind="ExternalInput")
    xt_dram = nc.dram_tensor("xt", [BLOC, D, N], bf16, kind="ExternalInput")
    out_dram = nc.dram_tensor("maxes", [P, BLOC * NT], f32, kind="ExternalOutput")

    with tile.TileContext(nc) as tc:
        with (
            tc.tile_pool(name="const", bufs=1) as cpool,
            tc.tile_pool(name="xin", bufs=2) as xpool,
            tc.tile_pool(name="xt", bufs=2) as xtpool,
            tc.tile_pool(name="stats", bufs=2) as spool,
            tc.tile_pool(name="scr", bufs=2) as scpool,
            tc.tile_pool(name="outp", bufs=1) as opool,
            tc.tile_pool(name="gpsum", bufs=3, space="PSUM") as gpool,
            tc.tile_pool(name="rpsum", bufs=1, space="PSUM") as rpool,
        ):
            identF = cpool.tile([P, P], f32)
            masks.make_identity(nc, identF[:])
            identB = cpool.tile([P, P], bf16)
            masks.make_identity(nc, identB[:])
            # -NEG on the diagonal (bf16), accumulated into G via PE matmul
            negbig = cpool.tile([P, P], bf16)
            nc.gpsimd.memset(negbig[:], 0.0)
            nc.gpsimd.affine_select(
                out=negbig[:],
                in_=negbig[:],
                compare_op=ALU.not_equal,
                fill=NEG,
                base=0,
                pattern=[[-1, P]],
                channel_multiplier=1,
            )
            # oneh[k, t, q] = 1.0 iff k == t ; lhsT slice t replicates row t
            oneh = cpool.tile([NT, NT, P], bf16)
            nc.gpsimd.memset(oneh[:], 0.0)
            nc.gpsimd.affine_select(
                out=oneh[:],
                in_=oneh[:],
                compare_op=ALU.not_equal,
                fill=1.0,
                base=0,
                pattern=[[-1, NT], [0, P]],
                channel_multiplier=1,
            )

            maxes = opool.tile([P, BLOC * NT], f32)
            xb_r = xb_dram.ap().rearrange("b (t p) d -> b p t d", p=P)
            xt_r = xt_dram.ap().rearrange("b (k p) n -> b p k n", p=P)

            # PE warm-up: the HAM clock gate keeps the PE at 1.2 GHz until
            # it has seen ~3.4us of sustained activity, and re-throttles
            # after ~3.4us idle. Dummy matmuls fill the otherwise-idle head
            # so the real matmuls run at 2.4 GHz from the start.
            warm_rhs = cpool.tile([P, 512], bf16)
            nc.gpsimd.memset(warm_rhs[:], 0.0)

            def warm(n):
                warm_ps = gpool.tile([P, N], f32, tag="G")
                for _ in range(n):
                    nc.tensor.matmul(warm_ps[:, :512], identB[:], warm_rhs[:])

            # Pin the ACT table set: the first activation picks the set, and
            # 'sqrt_and_others' covers every function this kernel uses
            # (Sqrt, Square, Copy) -- later activations then never swap.
            pin = cpool.tile([P, 1], f32)
            nc.gpsimd.memset(pin[:], 1.0)
            nc.scalar.activation(pin[:], pin[:], AF.Sqrt)

            # Prep is split into pieces so they can be emitted interleaved
            # with the previous batch's matmul/reduce tiles: each engine's
            # FIFO then sees next-batch prep work only after enough current
            # work to hide the prep latency.
            def prep_load(b, st, head=False):
                x_all = xpool.tile([P, NT, D], bf16, tag="x_all")
                if head:
                    q = NT // 4
                    for z in range(4):
                        nc.sync.dma_start(
                            x_all[:, z * q : (z + 1) * q],
                            xb_r[b][:, z * q : (z + 1) * q],
                        )
                else:
                    nc.sync.dma_start(x_all[:], xb_r[b])
                xt_all = xtpool.tile([P, KC, N], bf16, tag="xt_all")
                nc.sync.dma_start(xt_all[:], xt_r[b])
                st["x_all"], st["xt_all"] = x_all, xt_all

            def prep_ssq(b, st, head=False):
                x_all = st["x_all"]
                ssq = spool.tile([P, NT], f32, tag="ssq")
                ndve = 4 if head else 0  # head: split squares ACT || DVE
                for i in range(NT - ndve):
                    sq = scpool.tile([P, D], bf16, tag="sq")
                    nc.scalar.activation(
                        sq[:], x_all[:, i], AF.Square, accum_out=ssq[:, i : i + 1]
                    )
                for i in range(NT - ndve, NT):
                    # bf16 scratch keeps the DVE in its fast copy modes;
                    # the ssq accumulation itself stays fp32 in the reduce
                    sqf = scpool.tile([P, D], bf16, tag="sqf")
                    nc.vector.tensor_mul(sqf[:], x_all[:, i], x_all[:, i])
                    nc.vector.reduce_sum(
                        ssq[:, i : i + 1], sqf[:], axis=mybir.AxisListType.X
                    )
                # norm (with the x64 fp8 range scale folded into Sqrt's
                # input scale), then rinv on the DVE. Sqrt/Square/Copy all
                # live in the 'sqrt_and_others' ACT table set, pinned by the
                # dummy sqrt at kernel start -- no table swaps.
                nrm = spool.tile([P, NT], f32, tag="nrm")
                nc.scalar.activation(nrm[:], ssq[:], AF.Sqrt, scale=1.0 / 4096.0)
                rinv = spool.tile([P, NT], f32, tag="rinv")
                nc.vector.reciprocal(rinv[:], nrm[:])
                st["rinv"] = rinv

            def prep_rbc(b, st):
                # broadcast rinv (col-indexed) across all partitions:
                # rinvT[t, q] = rinv[q, t], rbc[p, t*P+q] = rinvT[t, q].
                # All-bf16 (f32 matmuls cost 4 cycles/column on the PE).
                rinv_bf = spool.tile([P, NT], bf16, tag="rinv_bf")
                nc.vector.tensor_copy(rinv_bf[:], st["rinv"][:])
                rbc_ps = rpool.tile([P, N], f32, tag="rbc")
                # transient bf16 [8,128] staging inside the f32 tile
                rinvT_ps = rbc_ps[:NT, :P // 2].bitcast(bf16)
                nc.tensor.matmul(rinvT_ps, rinv_bf[:], identB[:], is_transpose=True)
                rinvT = spool.tile([NT, P], bf16, tag="rinvT")
                nc.scalar.copy(rinvT[:], rinvT_ps)
                for t in range(NT):
                    nc.tensor.matmul(
                        rbc_ps[:, t * P : (t + 1) * P],
                        oneh[:, t, :],
                        rinvT[:],
                    )
                rbc = scpool.tile([P, N], bf16, tag="rbc_sb")
                nc.scalar.copy(rbc[:], rbc_ps[:])
                st["rbc"] = rbc

            def prep_scale(b, st):
                # column-normalize the transposed operand: xnT = xt * rbc
                xnT = xtpool.tile([P, KC, N], bf16, tag="xnT")
                for k in range(KC):
                    nc.vector.tensor_mul(xnT[:, k], st["xt_all"][:, k], st["rbc"][:])
                st["xnT"] = xnT

            def mm_tile(b, t, xnT):
                G = gpool.tile([P, N], f32, tag="G")
                hd = t // 4  # which 512-half holds the diagonal block
                for k in range(KC):
                    lhsT = xnT[:, k, t * P : (t + 1) * P]
                    for h in range(2):
                        nc.tensor.matmul(
                            G[:, h * 512 : (h + 1) * 512],
                            lhsT,
                            xnT[:, k, h * 512 : (h + 1) * 512],
                            start=(k == 0),
                            stop=(k == KC - 1 and h != hd),
                        )
                # mask the diagonal: G[diag block] += NEG * I
                nc.tensor.matmul(
                    G[:, t * P : (t + 1) * P],
                    identB[:],
                    negbig[:],
                    start=False,
                    stop=True,
                )
                nc.vector.reduce_max(
                    maxes[:, b * NT + t : b * NT + t + 1],
                    G[:, :],
                    axis=mybir.AxisListType.X,
                )

            # Head: fully prep batch 0, and get batch 1 through ssq, with
            # PE warm-up matmuls covering the otherwise PE-idle stretches.
            states = {b: {} for b in range(BLOC)}
            warm(10)
            prep_load(0, states[0], head=True)
            prep_ssq(0, states[0], head=True)
            prep_rbc(0, states[0])
            warm(12)
            prep_scale(0, states[0])
            if BLOC > 1:
                prep_load(1, states[1])
                prep_ssq(1, states[1])

            # Steady state: during batch b's matmul/reduce phase, load and
            # ssq batch b+2 (ACT has two phases of slack) and finish batch
            # b+1 (rbc broadcast + column scale) -- so the PE rolls from
            # phase to phase without an idle window.
            for b in range(BLOC):
                for t in range(NT):
                    if t == 0 and b + 2 < BLOC:
                        prep_load(b + 2, states[b + 2])
                    elif t == 1 and b + 1 < BLOC:
                        prep_rbc(b + 1, states[b + 1])
                    elif t == 2 and b + 1 < BLOC:
                        prep_scale(b + 1, states[b + 1])
                    elif t == 4 and b + 2 < BLOC:
                        prep_ssq(b + 2, states[b + 2])
                    mm_tile(b, t, states[b]["xnT"])

            nc.sync.dma_start(out_dram.ap(), maxes[:])

    nc.compile()
    return nc


def get_nc():
    if "nc" not in _CACHE:
        _CACHE["nc"] = build_nc()
    return _CACHE["nc"]


def shard_inputs(sparse_feats):
    import ml_dtypes

    x = np.ascontiguousarray(sparse_feats, dtype=np.float32).reshape(
        NCORES, BLOC, N, D
    )
    xb = x.astype(ml_dtypes.bfloat16)
    xt = np.ascontiguousarray(xb.transpose(0, 1, 3, 2))
    return [{"xb": xb[c], "xt": xt[c]} for c in range(NCORES)]


def finalize(m_all):
    """m_all: any array containing the 32768 per-row max cosine sims."""
    m = np.asarray(m_all, dtype=np.float64)
    t = np.maximum(2.0 - 2.0 * m, 0.0)
    dist = 0.5 * np.sqrt(t)
    return np.float32(-np.mean(np.log(dist + EPS)))


def run_on_hw(sparse_feats, trace=False, **kw):
    from concourse.bass_utils import run_bass_kernel_spmd

    nc = get_nc()
    res = run_bass_kernel_spmd(
        nc, shard_inputs(sparse_feats), list(range(NCORES)), trace=trace, **kw
    )
    m = np.stack([res.results[c]["maxes"] for c in range(NCORES)]) / 4096.0
    return finalize(m), res


def kernel(sparse_feats):
    loss, _ = run_on_hw(sparse_feats)
    return loss

